# revision 13
# baseline (speedup 1.0000x reference)
"""AugmentedLSTMCell on 8 TRN2 NeuronCores — data-parallel over batch.

Mixed fp8/bf16 precision:
  - The i/f/o/hw gate projections (both GEMMs) run as fp8e4 DoubleRow
    matmuls (256-deep contraction per instruction, 2x PE throughput).
    Inputs are scaled by SX (x,h) / SW (weights) before e4m3
    quantization; the ScalarE activation un-scales via its `scale`
    operand (exact power-of-two, no extra rounding).
  - The tanh candidate path (m) and the highway projection (hwp) — the
    two paths that dominate output error — stay bf16.
  Simulated end-to-end rel err: out 1.46e-2, mem 1.32e-2 (gate 2e-2).

Layout: feature-on-partition (transposed). Per core: B_loc=2048 rows.
  proj.T[j, b] = sum_e W[j, e] * in[b, e]
  bf16: lhsT = W.T blocks [128e, 128j], rhs = in.T [128e, 2048b]
  fp8 DoubleRow: lhsT = [128p, 2s, 128j], rhs = [128p, 2s, 512b] where
    contraction index = kk*256 + s*128 + p  (kk = double-k-tile 0..3)
  psum [128j, 2048b] accumulates Wi-proj + Ws-proj (fused add free).
  ScalarE applies scale (fp8 only) + per-feature bias + activation.
Host transposes outputs back to [B, H].
"""
import sys
import types

sys.path.insert(0, "/opt/trn_rl_repo")
sys.path.insert(0, "/root/.axon_site")

# Shim antenv.axon_hooks (missing on this image) so trace=True can profile.
if "antenv.axon_hooks" not in sys.modules:
    _hooks = types.ModuleType("antenv.axon_hooks")
    _state = {"hook": None}
    _hooks.set_axon_ntff_profile_hook = lambda h: _state.__setitem__("hook", h)
    _hooks.get_axon_ntff_profile_hook = lambda: _state["hook"]
    sys.modules["antenv.axon_hooks"] = _hooks
    try:
        from trn_agent_boot.trn_boot import _ntff_profile_via_ctypes

        _hooks.set_axon_ntff_profile_hook(
            _ntff_profile_via_ctypes("/opt/axon/libaxon_pjrt.so")
        )
    except Exception:
        pass

import numpy as np
import ml_dtypes

import concourse.bass as bass
import concourse.bacc as bacc
import concourse.mybir as mybir
from concourse import tile
from concourse.bass_utils import run_bass_kernel_spmd

BF16 = ml_dtypes.bfloat16
E4M3 = ml_dtypes.float8_e4m3

N_CORES = 8
B, E, H = 16384, 1024, 1024
BL = B // N_CORES          # 2048 batch rows per core
KT = E // 128              # 8 contraction k-tiles (bf16 path)
KT2 = E // 256             # 4 double-k-tiles (fp8 DoubleRow path)
NJI = 6 * H // 128         # 48 feature tiles of proj_in (bias layout)
NT = H // 128              # 8 H-slices
BC = 512                   # matmul moving free dim (one PSUM bank)
NBC = BL // BC             # batch chunks per matmul group

SX = 16.0                  # x/h fp8 scale
SW = 128.0                 # weight fp8 scale
INV = 1.0 / (SX * SW)      # exact 2^-11

# Gate row-blocks in Wi/Ws: i=0, f=1, m=2, o=3, hw=4, hwp=5 (Wi only).
G8 = [0, 1, 3, 4]          # fp8 gates (i, f, o, hw)

AF = mybir.ActivationFunctionType
DR = mybir.MatmulPerfMode.DoubleRow


def build_nc():
    nc = bacc.Bacc(None, target_bir_lowering=False)
    f32, bf16, f8 = mybir.dt.float32, mybir.dt.bfloat16, mybir.dt.float8e4

    x8 = nc.declare_dram_parameter("x8", [KT2, 128, 2, BL], f8, isOutput=False)
    h8 = nc.declare_dram_parameter("h8", [KT2, 128, 2, BL], f8, isOutput=False)
    xT = nc.declare_dram_parameter("xT", [E, BL], bf16, isOutput=False)
    hT = nc.declare_dram_parameter("hT", [H, BL], bf16, isOutput=False)
    cT = nc.declare_dram_parameter("cT", [H, BL], bf16, isOutput=False)
    # fp8 weights: [gate(i,f,o,hw)*8+t, p, kk, s, m]
    w8i = nc.declare_dram_parameter("w8i", [32, 128, KT2, 2, 128], f8, isOutput=False)
    w8s = nc.declare_dram_parameter("w8s", [32, 128, KT2, 2, 128], f8, isOutput=False)
    # bf16 weights (baseline packing): m path (Wi, Ws) + hwp (Wi)
    wbmi = nc.declare_dram_parameter("wbmi", [NT, 128, E], bf16, isOutput=False)
    wbms = nc.declare_dram_parameter("wbms", [NT, 128, H], bf16, isOutput=False)
    wbh = nc.declare_dram_parameter("wbh", [NT, 128, E], bf16, isOutput=False)
    bias = nc.declare_dram_parameter("bias", [128, NJI], f32, isOutput=False)
    outT = nc.declare_dram_parameter("outT", [H, BL], f32, isOutput=True)
    memT = nc.declare_dram_parameter("memT", [H, BL], f32, isOutput=True)

    with tile.TileContext(nc) as tc:
        with (
            tc.tile_pool(name="resident", bufs=1) as resident,
            tc.tile_pool(name="w8pool", bufs=16) as w8pool,
            tc.tile_pool(name="wbpool", bufs=6) as wbpool,
            tc.tile_pool(name="cpool", bufs=2) as cpool,
            tc.tile_pool(name="psum", bufs=2, space="PSUM") as psum_pool,
            tc.tile_pool(name="gates", bufs=8) as gate_pool,
            tc.tile_pool(name="tmp", bufs=4) as tmp_pool,
            tc.tile_pool(name="outp", bufs=2) as out_pool,
        ):
            def split_last(dst, src, nsplit, eng=None):
                eng = eng or nc.sync
                n = dst.shape[-1]
                per = -(-n // nsplit)
                for q in range(nsplit):
                    sl = slice(q * per, min((q + 1) * per, n))
                    if sl.start >= n:
                        break
                    if len(dst.shape) == 3:
                        eng.dma_start(dst[:, :, sl], src[:, :, sl])
                    else:
                        eng.dma_start(dst[:, sl], src[:, sl])

            def split_w8(dst, src, eng):
                # split on the kk dim: contiguous 512B lines per piece
                eng.dma_start(dst[:, 0:2], src[:, 0:2])
                eng.dma_start(dst[:, 2:4], src[:, 2:4])

            bias_sb = resident.tile([128, NJI], f32, tag="bias")

            # Residents: fp8 x/h (DoubleRow layout) + bf16 xT/hT.
            x8sb = [
                resident.tile([128, 2, BL], f8, tag=f"x8{k}", name=f"x8{k}")
                for k in range(KT2)
            ]
            h8sb = [
                resident.tile([128, 2, BL], f8, tag=f"h8{k}", name=f"h8{k}")
                for k in range(KT2)
            ]
            xt_k = [
                resident.tile([128, BL], bf16, tag=f"xt{k}", name=f"xt{k}")
                for k in range(KT)
            ]
            ht_k = [
                resident.tile([128, BL], bf16, tag=f"ht{k}", name=f"ht{k}")
                for k in range(KT)
            ]

            # --- startup loads, strict need-order -------------------------
            # Per-dma_start issue/descriptor-gen cost is ~0.6us serialized
            # on the issuing engine, so spread issues over all three
            # DMA-capable engines (gpsimd, sync, scalar): i-gate t=0 weights
            # first (first matmuls), then fp8 x/h, then remaining t=0
            # weights, then the bf16 residents.
            w8i_0 = w8pool.tile([128, KT2, 2, 128], f8, tag="w8")
            split_w8(w8i_0, w8i[0 * NT + 0], nc.gpsimd)
            w8s_0 = w8pool.tile([128, KT2, 2, 128], f8, tag="w8")
            split_w8(w8s_0, w8s[0 * NT + 0], nc.sync)
            split_last(x8sb[0], x8[0], 4, eng=nc.gpsimd)
            split_last(h8sb[0], h8[0], 4, eng=nc.sync)
            split_last(x8sb[1], x8[1], 4, eng=nc.scalar)
            split_last(x8sb[2], x8[2], 4, eng=nc.gpsimd)
            split_last(x8sb[3], x8[3], 4, eng=nc.scalar)
            split_last(h8sb[1], h8[1], 4, eng=nc.sync)
            split_last(h8sb[2], h8[2], 4, eng=nc.scalar)
            split_last(h8sb[3], h8[3], 4, eng=nc.sync)
            nc.sync.dma_start(bias_sb[:], bias[:])
            # rest of t=0 weights (f, o, hw fp8 pairs + hwp/m bf16)
            w8_pre = {(0, 0): (w8i_0, w8s_0)}
            for gi in range(1, 4):
                wi_t = w8pool.tile([128, KT2, 2, 128], f8, tag="w8")
                split_w8(wi_t, w8i[gi * NT + 0], nc.gpsimd)
                ws_t = w8pool.tile([128, KT2, 2, 128], f8, tag="w8")
                split_w8(ws_t, w8s[gi * NT + 0], nc.sync)
                w8_pre[(gi, 0)] = (wi_t, ws_t)
            wb_pre = {}
            wbh_0 = wbpool.tile([128, E], bf16, tag="wb")
            split_last(wbh_0, wbh[0], 2, eng=nc.scalar)
            wbmi_0 = wbpool.tile([128, E], bf16, tag="wb")
            split_last(wbmi_0, wbmi[0], 2, eng=nc.scalar)
            wbms_0 = wbpool.tile([128, H], bf16, tag="wb")
            split_last(wbms_0, wbms[0], 2, eng=nc.scalar)
            wb_pre[0] = (wbh_0, wbmi_0, wbms_0)
            # bf16 residents: x first (hwp needs only x), then h (m path),
            # round-robined across the three issuing engines.
            engs = [nc.gpsimd, nc.sync, nc.scalar]
            for k in range(KT):
                split_last(xt_k[k], xT[k * 128 : (k + 1) * 128, :], 3,
                           eng=engs[k % 3])
            for k in range(KT):
                split_last(ht_k[k], hT[k * 128 : (k + 1) * 128, :], 3,
                           eng=engs[(k + 1) % 3])
            ct_pre = cpool.tile([128, BL], bf16, tag="c")
            split_last(ct_pre, cT[0:128, :], 2, eng=nc.gpsimd)

            # --- feature tile builders -----------------------------------
            def fp8_tile(gi, t, w_pair=None, chunk_act=1, bc0=0, bc1=NBC):
                """fp8 DoubleRow fused tile -> activated gate (bf16)."""
                jt = G8[gi] * NT + t
                if w_pair is None:
                    w_i = w8pool.tile([128, KT2, 2, 128], f8, tag="w8")
                    nc.gpsimd.dma_start(w_i[:], w8i[gi * NT + t])
                    w_s = w8pool.tile([128, KT2, 2, 128], f8, tag="w8")
                    nc.sync.dma_start(w_s[:], w8s[gi * NT + t])
                else:
                    w_i, w_s = w_pair
                width = (bc1 - bc0) * BC
                ps = psum_pool.tile([128, width], mybir.dt.float32, tag="ps")
                for kk in range(KT2):
                    for bc in range(bc0, bc1):
                        lo = (bc - bc0) * BC
                        nc.tensor.matmul(
                            ps[:, lo : lo + BC],
                            w_i[:, kk],
                            x8sb[kk][:, :, bc * BC : (bc + 1) * BC],
                            start=(kk == 0),
                            stop=False,
                            perf_mode=DR,
                        )
                for kk in range(KT2):
                    for bc in range(bc0, bc1):
                        lo = (bc - bc0) * BC
                        nc.tensor.matmul(
                            ps[:, lo : lo + BC],
                            w_s[:, kk],
                            h8sb[kk][:, :, bc * BC : (bc + 1) * BC],
                            start=False,
                            stop=(kk == KT2 - 1),
                            perf_mode=DR,
                        )
                g = gate_pool.tile([128, width], mybir.dt.bfloat16, tag="g")
                cw = width // chunk_act
                for a in range(chunk_act):
                    sl = slice(a * cw, (a + 1) * cw)
                    nc.scalar.activation(
                        g[:, sl], ps[:, sl], AF.Sigmoid,
                        bias=bias_sb[:, jt : jt + 1], scale=INV,
                    )
                return g

            def bf16_tile(t, func, kind, w_pre=None):
                """bf16 feature tile: kind 'm' (fused) or 'hwp' (x only)."""
                if kind == "m":
                    jt = 2 * NT + t
                    if w_pre is None:
                        w_i = wbpool.tile([128, E], bf16, tag="wb")
                        nc.sync.dma_start(w_i[:], wbmi[t])
                        w_s = wbpool.tile([128, H], bf16, tag="wb")
                        nc.sync.dma_start(w_s[:], wbms[t])
                    else:
                        w_i, w_s = w_pre
                else:
                    jt = 5 * NT + t
                    if w_pre is None:
                        w_i = wbpool.tile([128, E], bf16, tag="wb")
                        nc.sync.dma_start(w_i[:], wbh[t])
                    else:
                        w_i = w_pre
                    w_s = None
                ps = psum_pool.tile([128, BL], mybir.dt.float32, tag="ps")
                for k in range(KT):
                    lhsT = w_i[:, k * 128 : (k + 1) * 128]
                    for bc in range(NBC):
                        nc.tensor.matmul(
                            ps[:, bc * BC : (bc + 1) * BC],
                            lhsT,
                            xt_k[k][:, bc * BC : (bc + 1) * BC],
                            start=(k == 0),
                            stop=(w_s is None and k == KT - 1),
                        )
                if w_s is not None:
                    for k in range(KT):
                        lhsT = w_s[:, k * 128 : (k + 1) * 128]
                        for bc in range(NBC):
                            nc.tensor.matmul(
                                ps[:, bc * BC : (bc + 1) * BC],
                                lhsT,
                                ht_k[k][:, bc * BC : (bc + 1) * BC],
                                start=False,
                                stop=(k == KT - 1),
                            )
                g = gate_pool.tile([128, BL], mybir.dt.bfloat16, tag="g")
                nc.scalar.activation(
                    g[:], ps[:], func, bias=bias_sb[:, jt : jt + 1]
                )
                return g

            mult, addop, subop = (
                mybir.AluOpType.mult,
                mybir.AluOpType.add,
                mybir.AluOpType.subtract,
            )

            # --- main loop over H-slices ---------------------------------
            for t in range(NT):
                # prefetch next H-slice weights + c (lands during this slice)
                if t + 1 < NT:
                    for gi in range(4):
                        wi_t = w8pool.tile([128, KT2, 2, 128], f8, tag="w8")
                        nc.gpsimd.dma_start(wi_t[:], w8i[gi * NT + t + 1])
                        ws_t = w8pool.tile([128, KT2, 2, 128], f8, tag="w8")
                        nc.sync.dma_start(ws_t[:], w8s[gi * NT + t + 1])
                        w8_pre[(gi, t + 1)] = (wi_t, ws_t)
                    wbh_t = wbpool.tile([128, E], bf16, tag="wb")
                    nc.gpsimd.dma_start(wbh_t[:], wbh[t + 1])
                    wbmi_t = wbpool.tile([128, E], bf16, tag="wb")
                    nc.sync.dma_start(wbmi_t[:], wbmi[t + 1])
                    wbms_t = wbpool.tile([128, H], bf16, tag="wb")
                    nc.sync.dma_start(wbms_t[:], wbms[t + 1])
                    wb_pre[t + 1] = (wbh_t, wbmi_t, wbms_t)

                ct = ct_pre
                if t + 1 < NT:
                    ct_pre = cpool.tile([128, BL], bf16, tag="c")
                    nc.sync.dma_start(
                        ct_pre[:], cT[(t + 1) * 128 : (t + 2) * 128, :]
                    )

                i_g = fp8_tile(0, t, w_pair=w8_pre.pop((0, t)))
                f_g = fp8_tile(1, t, w_pair=w8_pre.pop((1, t)))
                o_g = fp8_tile(2, t, w_pair=w8_pre.pop((2, t)))
                hw_g = None
                if t < NT - 1:
                    # hw 4th: keeps the sigmoid table loaded and, at t=0,
                    # gives the bf16 residents more time to stream in.
                    hw_g = fp8_tile(3, t, w_pair=w8_pre.pop((3, t)), chunk_act=4)
                wbh_t, wbmi_t, wbms_t = wb_pre.pop(t)
                hwp = bf16_tile(t, AF.Identity, "hwp", w_pre=wbh_t)
                m_g = bf16_tile(t, AF.Tanh, "m", w_pre=(wbmi_t, wbms_t))

                t1 = tmp_pool.tile([128, BL], bf16, tag="tmp")
                nc.vector.tensor_tensor(t1[:], i_g[:], m_g[:], mult)
                t2 = tmp_pool.tile([128, BL], bf16, tag="tmp")
                nc.vector.tensor_tensor(t2[:], f_g[:], ct[:], mult)
                mem = out_pool.tile([128, BL], mybir.dt.float32, tag="mem")
                nc.vector.tensor_tensor(mem[:], t1[:], t2[:], addop)
                for q in range(4):
                    sl = slice(q * (BL // 4), (q + 1) * (BL // 4))
                    nc.scalar.dma_start(memT[t * 128 : (t + 1) * 128, sl], mem[:, sl])

                tmem = tmp_pool.tile([128, BL], bf16, tag="tmp")
                nc.scalar.activation(tmem[:], mem[:], AF.Tanh)
                outp = tmp_pool.tile([128, BL], bf16, tag="tmp")
                nc.vector.tensor_tensor(outp[:], o_g[:], tmem[:], mult)
                # out = hwp + hw*(outp - hwp), chunked so the tail after the
                # final hw matmuls pipelines with the output DMA.
                u = tmp_pool.tile([128, BL], bf16, tag="tmp")
                nc.vector.tensor_tensor(u[:], outp[:], hwp[:], subop)

                def blend(hw_tile, col0, ncols, nchunk):
                    ec = ncols // nchunk
                    for e in range(nchunk):
                        sl = slice(col0 + e * ec, col0 + (e + 1) * ec)
                        lsl = slice(e * ec, (e + 1) * ec)
                        v = tmp_pool.tile([128, ec], bf16, tag="v")
                        nc.vector.tensor_tensor(v[:], hw_tile[:, lsl], u[:, sl], mult)
                        outf = out_pool.tile([128, ec], mybir.dt.float32, tag="out")
                        nc.vector.tensor_tensor(outf[:], v[:], hwp[:, sl], addop)
                        nc.scalar.dma_start(outT[t * 128 : (t + 1) * 128, sl], outf[:])

                if t < NT - 1:
                    blend(hw_g, 0, BL, 4)
                else:
                    # Last slice: split hw in half so the first half's
                    # blend+DMA overlaps the second half's matmuls; fine
                    # blend chunks (256 cols, 128KB DMAs) shorten the drain.
                    wpair = w8_pre.pop((3, t))
                    for half in range(2):
                        hw_h = fp8_tile(
                            3, t, w_pair=wpair, chunk_act=2,
                            bc0=2 * half, bc1=2 * half + 2,
                        )
                        blend(hw_h, half * (BL // 2), BL // 2, 4 if half else 2)

    nc.compile()
    return nc


_NC_CACHE = None


def _get_nc():
    global _NC_CACHE
    if _NC_CACHE is None:
        _NC_CACHE = build_nc()
    return _NC_CACHE


def _pack_w_bf16(Wblk):
    # Wblk [NT*128 j, K e] -> [NT, 128 p, K] with [t, p, k*128+m] = W[t*128+m, k*128+p]
    K = Wblk.shape[1]
    kt = K // 128
    return np.ascontiguousarray(
        Wblk.reshape(NT, 128, kt, 128).transpose(0, 3, 2, 1).reshape(NT, 128, K)
    ).astype(BF16)


def _pack_w_fp8(Wblk):
    # Wblk [1024 j, 1024 e] -> [NT, 128 p, KT2, 2, 128 m] with
    # [t, p, kk, s, m] = W[t*128+m, kk*256+s*128+p] * SW, e4m3-quantized
    a = Wblk.reshape(NT, 128, KT2, 2, 128)      # [t, m, kk, s, p]
    a = a.transpose(0, 4, 2, 3, 1)              # [t, p, kk, s, m]
    return np.ascontiguousarray(a * SW).astype(E4M3)


def _pack_act_fp8(a_sl):
    # a_sl [BL, E] -> [KT2, 128 p, 2 s, BL] fp8 with [kk,p,s,b] = a[b, kk*256+s*128+p]*SX
    aT = np.ascontiguousarray(a_sl.astype(np.float32).T * SX)   # [E, BL]
    aT = aT.reshape(KT2, 2, 128, BL).transpose(0, 2, 1, 3)      # [kk, p, s, b]
    return np.ascontiguousarray(aT).astype(E4M3)


def prepare_in_maps(x, h, c, Wi, bi, Ws, bs):
    Wi = np.asarray(Wi, np.float32)
    Ws = np.asarray(Ws, np.float32)
    w8i_p = np.concatenate([_pack_w_fp8(Wi[g * H : (g + 1) * H]) for g in G8])
    w8s_p = np.concatenate([_pack_w_fp8(Ws[g * H : (g + 1) * H]) for g in G8])
    wbmi_p = _pack_w_bf16(Wi[2 * H : 3 * H])
    wbms_p = _pack_w_bf16(Ws[2 * H : 3 * H])
    wbh_p = _pack_w_bf16(Wi[5 * H : 6 * H])
    bias_comb = np.concatenate(
        [np.asarray(bi[: 5 * H], np.float32) + np.asarray(bs, np.float32),
         np.asarray(bi[5 * H :], np.float32)]
    )
    bias_pack = np.ascontiguousarray(bias_comb.reshape(NJI, 128).T).astype(np.float32)

    in_maps = []
    for i in range(N_CORES):
        s = slice(i * BL, (i + 1) * BL)
        x_sl = np.asarray(x[s], np.float32)
        h_sl = np.asarray(h[s], np.float32)
        in_maps.append(
            {
                "x8": _pack_act_fp8(x_sl),
                "h8": _pack_act_fp8(h_sl),
                "xT": np.ascontiguousarray(x_sl.T).astype(BF16),
                "hT": np.ascontiguousarray(h_sl.T).astype(BF16),
                "cT": np.ascontiguousarray(np.asarray(c[s], np.float32).T).astype(BF16),
                "w8i": w8i_p,
                "w8s": w8s_p,
                "wbmi": wbmi_p,
                "wbms": wbms_p,
                "wbh": wbh_p,
                "bias": bias_pack,
            }
        )
    return in_maps


def run(in_maps, trace=False):
    nc = _get_nc()
    res = run_bass_kernel_spmd(nc, in_maps, core_ids=list(range(N_CORES)), trace=trace)
    out = np.empty((B, H), np.float32)
    mem = np.empty((B, H), np.float32)
    for i in range(N_CORES):
        s = slice(i * BL, (i + 1) * BL)
        out[s] = res.results[i]["outT"].T
        mem[s] = res.results[i]["memT"].T
    return (out, mem), res


def kernel(x, h, c, Wi, bi, Ws, bs):
    in_maps = prepare_in_maps(x, h, c, Wi, bi, Ws, bs)
    (out, mem), _ = run(in_maps, trace=False)
    return out, mem
